# revision 1
# baseline (speedup 1.0000x reference)
"""Trainium2 Bass kernel for a 3D-gaussian-splatting rasterizer.

Pipeline:
  host (numpy, O(N) work): quaternion -> cov3D -> EWA cov2D -> conic,
    projection, depth sort, per-16x16-tile culling, per-core work packing.
  device (8 NeuronCores, SPMD): per [128-gaussian x 256-pixel] block,
    power = coef-matmul over a 6-term pixel basis; alpha = exp(power);
    threshold-mask (fused scalar_tensor_tensor); L = ln(1-alpha);
    depth-ordered transmittance via strict-lower-triangular cumsum matmul
    + carry-broadcast matmul; weights = exp(cum) * alpha; color matmul
    -> [3,256] partials DMA'd out.
  host: sum block partials per tile, add residual-transmittance * bg,
    scatter tiles into the [3,128,128] image.
"""

import os
import numpy as np

N_CORES = 8
H = W = 128
TS = 16            # pixel tile is TS x TS
PIX = TS * TS      # 256 pixels per tile (matmul free dim)
KB = 128           # gaussians per block (partition dim)
TANFOV = 0.5
FOCAL = W / (2.0 * TANFOV)   # 128.0
ZNEAR = 0.2
ALPHA_MIN = 1.0 / 255.0
NEG_BIG = -1.0e9

_compiled_cache = {}


# ----------------------------------------------------------------------------
# Host-side per-gaussian preprocessing (numpy, O(N))
# ----------------------------------------------------------------------------

def _preprocess(means3D, opacities, colors_precomp, scales, rotations, viewmatrix):
    q = rotations / np.linalg.norm(rotations, axis=-1, keepdims=True)
    r, x, y, z = q[:, 0], q[:, 1], q[:, 2], q[:, 3]
    R = np.stack([
        1 - 2 * (y * y + z * z), 2 * (x * y - r * z), 2 * (x * z + r * y),
        2 * (x * y + r * z), 1 - 2 * (x * x + z * z), 2 * (y * z - r * x),
        2 * (x * z - r * y), 2 * (y * z + r * x), 1 - 2 * (x * x + y * y),
    ], axis=-1).reshape(-1, 3, 3)
    M = R * scales[:, None, :]
    cov3D = np.einsum('nij,nkj->nik', M, M)

    Wm = viewmatrix[:3, :3]
    t = means3D @ Wm.T + viewmatrix[:3, 3]
    tz = t[:, 2]
    lim = 1.3 * TANFOV
    txz = np.clip(t[:, 0] / tz, -lim, lim) * tz
    tyz = np.clip(t[:, 1] / tz, -lim, lim) * tz
    zero = np.zeros_like(tz)
    fx = fy = FOCAL
    J = np.stack([
        np.stack([fx / tz, zero, -fx * txz / (tz * tz)], axis=-1),
        np.stack([zero, fy / tz, -fy * tyz / (tz * tz)], axis=-1),
    ], axis=1)
    T = np.einsum('nij,jk->nik', J, Wm)
    cov2D = np.einsum('nij,njk,nlk->nil', T, cov3D, T)
    a = cov2D[:, 0, 0] + 0.3
    b = cov2D[:, 0, 1]
    c = cov2D[:, 1, 1] + 0.3
    det = a * c - b * b
    det_safe = np.where(det > 0, det, 1.0)
    conA, conB, conC = c / det_safe, -b / det_safe, a / det_safe
    px = fx * t[:, 0] / tz + (W - 1) * 0.5
    py = fy * t[:, 1] / tz + (H - 1) * 0.5
    valid = (det > 0) & (tz > ZNEAR)
    opac = opacities[:, 0]

    # bounding half-widths of the {alpha >= ALPHA_MIN} ellipse:
    # power = -0.5 d^T Q d with Q = [[conA,conB],[conB,conC]] = cov2D^-1, so
    # max|dx| over {d^T Q d <= 2*ell} is sqrt(2*ell*(Q^-1)_00) = sqrt(2*ell*a)
    ell = np.log(np.maximum(opac * 255.0, 1.0 + 1e-7))
    rx = np.where(valid, np.sqrt(np.maximum(2 * ell * a, 0.0)), 0.0)
    ry = np.where(valid, np.sqrt(np.maximum(2 * ell * c, 0.0)), 0.0)

    order = np.argsort(tz, kind='stable')
    return dict(conA=conA, conB=conB, conC=conC, px=px, py=py, opac=opac,
                cols=colors_precomp, valid=valid, rx=rx, ry=ry, order=order)


def _pack_work(pre):
    """Cull gaussians per tile, pack (tile, block) slots onto 8 cores."""
    NT = H // TS
    order = pre['order']
    valid = pre['valid'][order]
    px, py = pre['px'][order], pre['py'][order]
    rx, ry = pre['rx'][order], pre['ry'][order]

    tiles = []   # (ti, tj, idx_sorted_gaussians)
    for ti in range(NT):
        for tj in range(NT):
            ylo, yhi = ti * TS, ti * TS + TS - 1
            xlo, xhi = tj * TS, tj * TS + TS - 1
            hit = valid & (px + rx >= xlo) & (px - rx <= xhi) \
                        & (py + ry >= ylo) & (py - ry <= yhi)
            idx = np.nonzero(hit)[0]
            tiles.append((ti, tj, idx))

    nblocks = [max(1, -(-len(idx) // KB)) for _, _, idx in tiles]
    loads = [0] * N_CORES
    core_tiles = [[] for _ in range(N_CORES)]
    for k in np.argsort([-b for b in nblocks], kind='stable'):
        core = int(np.argmin(loads))
        loads[core] += nblocks[k]
        core_tiles[core].append(k)
    B = max(loads)

    # slots[core] = list of (tile_index or -1, block_index, is_first)
    slots = [[] for _ in range(N_CORES)]
    for core in range(N_CORES):
        for k in core_tiles[core]:
            for b in range(nblocks[k]):
                slots[core].append((k, b, b == 0))
        while len(slots[core]) < B:
            slots[core].append((-1, 0, True))
    return tiles, slots, B


def _build_core_arrays(pre, tiles, slots_core, B):
    """Build the per-core DRAM input arrays."""
    order = pre['order']
    conA = pre['conA'][order]; conB = pre['conB'][order]; conC = pre['conC'][order]
    px = pre['px'][order]; py = pre['py'][order]
    opac = pre['opac'][order]; cols = pre['cols'][order]

    coef = np.zeros((6, B * KB), np.float32)
    colsT = np.zeros((KB, B * 4), np.float32)   # 4-col stride: bf16 word align
    maskrow = np.zeros((1, B * KB), np.float32)
    coef[5, :] = NEG_BIG

    for s, (k, b, is_first) in enumerate(slots_core):
        if k < 0:
            continue
        ti, tj, idx = tiles[k]
        xc = tj * TS + (TS - 1) * 0.5
        yc = ti * TS + (TS - 1) * 0.5
        seg = idx[b * KB:(b + 1) * KB]
        n = len(seg)
        A, Bc, C = conA[seg], conB[seg], conC[seg]
        pxr = px[seg] - xc
        pyr = py[seg] - yc
        sl = slice(s * KB, s * KB + n)
        coef[0, sl] = -0.5 * A
        coef[1, sl] = -0.5 * C
        coef[2, sl] = -Bc
        coef[3, sl] = A * pxr + Bc * pyr
        coef[4, sl] = C * pyr + Bc * pxr
        coef[5, sl] = -0.5 * (A * pxr * pxr + C * pyr * pyr) \
            - Bc * pxr * pyr + np.log(opac[seg])
        colsT[:n, s * 4:s * 4 + 3] = cols[seg]
        if not is_first:
            maskrow[0, s * KB:(s + 1) * KB] = 1.0
    return coef, colsT, maskrow


def _make_basis():
    c = np.arange(TS, dtype=np.float32) - (TS - 1) * 0.5
    r = np.arange(TS, dtype=np.float32) - (TS - 1) * 0.5
    ys, xs = np.meshgrid(r, c, indexing='ij')   # [r, c]
    xs = xs.reshape(-1); ys = ys.reshape(-1)
    basis = np.stack([xs * xs, ys * ys, xs * ys, xs, ys, np.ones_like(xs)])
    return np.ascontiguousarray(basis, np.float32)   # [6, 256]


# ----------------------------------------------------------------------------
# Device program
# ----------------------------------------------------------------------------

def _build_program(B):
    from contextlib import ExitStack
    import concourse.bass as bass
    import concourse.tile as tile
    from concourse import mybir, bacc

    f32 = mybir.dt.float32
    f32r = mybir.dt.float32r
    bf16 = mybir.dt.bfloat16
    AF = mybir.ActivationFunctionType
    OP = mybir.AluOpType

    class _BaccOneActSet(bacc.Bacc):
        # Pin Exp/Ln to the one table set containing both, so the scalar
        # engine doesn't reload activation tables (~1.3us) per call.
        def insert_act_table_loads(self):
            from concourse.hw_specs import get_activation_tables
            from concourse.bacc import _bass_rust
            AF = mybir.ActivationFunctionType
            tables = []
            for name, fns in get_activation_tables(self.m.arch).items():
                if name != 'natural_log_exp_and_others':
                    fns = fns - {AF.Exp, AF.Ln}
                tables.append((name, fns))
            _bass_rust.insert_act_table_loads(self, tables)

    nc = _BaccOneActSet(None)
    W1 = KB + 1                        # ustrict | onescol
    W2 = B * KB + PIX                  # coef | basis
    blob1_d = nc.declare_dram_parameter("blob1", [KB, W1], f32r, isOutput=False)
    blob2_d = nc.declare_dram_parameter("blob2", [6, W2], f32r, isOutput=False)
    cols_d = nc.declare_dram_parameter("cols", [KB, 4 * B], bf16, isOutput=False)
    mask_d = nc.declare_dram_parameter("maskrow", [1, B], f32r, isOutput=False)
    orgb_d = nc.declare_dram_parameter("orgb", [3, B * PIX], f32, isOutput=True)
    ocarry_d = nc.declare_dram_parameter("ocarry", [1, B * PIX], f32, isOutput=True)

    groups = [(g, min(g + 2, B)) for g in range(0, B, 2)]

    with ExitStack() as ctx:
        tc = ctx.enter_context(tile.TileContext(
            nc, linearize=bool(int(os.environ.get("GR_LINEARIZE", "0")))))
        const_pool = ctx.enter_context(tc.tile_pool(name="const", bufs=1))
        sb = ctx.enter_context(tc.tile_pool(name="work", bufs=3))
        ps = ctx.enter_context(tc.tile_pool(name="psum", bufs=2, space="PSUM"))

        blob1_sb = const_pool.tile([KB, W1], f32r)
        blob2_sb = const_pool.tile([6, W2], f32r)
        cols_sb = const_pool.tile([KB, 4 * B], bf16)
        mask_sb = const_pool.tile([1, B], f32r)
        carry_sb = const_pool.tile([1, PIX], f32r)

        u_sb = blob1_sb[:, 0:KB]                            # [KB, KB] f32r
        ones_sb = blob1_sb[:, KB:KB + 1]                    # [KB, 1] f32r
        coef_sb = blob2_sb[:, 0:B * KB]
        basis_sb = blob2_sb[:, B * KB:B * KB + PIX]

        nc.gpsimd.dma_start(blob1_sb[:], blob1_d[:])
        nc.gpsimd.dma_start(blob2_sb[:], blob2_d[:])
        nc.gpsimd.dma_start(cols_sb[:], cols_d[:])
        nc.gpsimd.dma_start(mask_sb[:], mask_d[:])
        nc.vector.memset(carry_sb[:].bitcast(f32), 0.0)

        for g0, g1 in groups:
            nb = g1 - g0
            FD = nb * PIX
            P2 = ps.tile([KB, 512], f32, tag="power")
            for i in range(nb):
                b = g0 + i
                nc.tensor.matmul(
                    P2[:, i * PIX:(i + 1) * PIX],
                    lhsT=coef_sb[:, b * KB:(b + 1) * KB],
                    rhs=basis_sb,
                    start=True, stop=True)
            A2 = sb.tile([KB, 512], f32, tag="A2")
            nc.scalar.activation(A2[:, :FD], P2[:, :FD], AF.Exp)
            aM2 = sb.tile([KB, 512], f32, tag="aM2")
            nc.vector.scalar_tensor_tensor(
                aM2[:, :FD], A2[:, :FD], float(ALPHA_MIN), A2[:, :FD],
                OP.is_ge, OP.mult)
            L2 = sb.tile([KB, 512], f32r, tag="L2")
            nc.scalar.activation(L2[:, :FD], aM2[:, :FD], AF.Ln,
                                 bias=1.0, scale=-1.0)
            C2 = ps.tile([KB, 512], f32, tag="cum")
            for i in range(nb):
                b = g0 + i
                sl = slice(i * PIX, (i + 1) * PIX)
                nc.tensor.matmul(
                    C2[:, sl],
                    lhsT=u_sb,
                    rhs=L2[:, sl],
                    start=True, stop=False)
                nc.tensor.matmul(
                    C2[:, sl],
                    lhsT=mask_sb[:, b:b + 1].broadcast_to([1, KB]),
                    rhs=carry_sb[:],
                    start=False, stop=True)
                # carry_new = m[b] * carry_old + sum_k L[k]: column-sum via
                # ones matmul into a [1, PIX] psum row, then one fused DVE op
                S2 = ps.tile([1, 512], f32, tag="carrysum")
                nc.tensor.matmul(S2[:, :PIX], lhsT=ones_sb,
                                 rhs=L2[:, sl], start=True, stop=True)
                nc.vector.scalar_tensor_tensor(
                    carry_sb[:], carry_sb[:], mask_sb[:, b:b + 1],
                    S2[:, :PIX], OP.mult, OP.add)
                nc.sync.dma_start(ocarry_d[:, b * PIX:(b + 1) * PIX],
                                  carry_sb[:].bitcast(f32))
            T2 = sb.tile([KB, 512], f32, tag="T2")
            nc.scalar.activation(T2[:, :FD], C2[:, :FD], AF.Exp)
            W2 = sb.tile([KB, 512], bf16, tag="W2")
            nc.gpsimd.tensor_mul(W2[:, :FD], T2[:, :FD], aM2[:, :FD])
            R2 = ps.tile([4, 512], f32, tag="rgb")
            for i in range(nb):
                b = g0 + i
                sl = slice(i * PIX, (i + 1) * PIX)
                nc.tensor.matmul(
                    R2[:, sl],
                    lhsT=cols_sb[:, b * 4:(b + 1) * 4],
                    rhs=W2[:, sl],
                    start=True, stop=True)
            Rsb = sb.tile([3, 512], f32, tag="rgbsb")
            nc.vector.tensor_copy(Rsb[:, :FD], R2[0:3, :FD])
            nc.sync.dma_start(orgb_d[:, g0 * PIX:g0 * PIX + FD], Rsb[:, :FD])

    nc.compile()
    return nc


# ----------------------------------------------------------------------------
# Entry point
# ----------------------------------------------------------------------------

def kernel(means3D, means2D, opacities, colors_precomp, scales, rotations,
           bg, viewmatrix):
    means3D = np.asarray(means3D, np.float32)
    opacities = np.asarray(opacities, np.float32)
    colors_precomp = np.asarray(colors_precomp, np.float32)
    scales = np.asarray(scales, np.float32)
    rotations = np.asarray(rotations, np.float32)
    bg = np.asarray(bg, np.float32)
    viewmatrix = np.asarray(viewmatrix, np.float32)

    pre = _preprocess(means3D, opacities, colors_precomp, scales, rotations,
                      viewmatrix)
    tiles, slots, B = _pack_work(pre)

    basis = _make_basis()
    # U[k,j]=1 for k<j: matmul out[j,p] = sum_k U[k,j] L[k,p] = sum_{k<j} L[k,p]
    ustrict = np.ascontiguousarray(np.triu(np.ones((KB, KB), np.float32), 1))

    import ml_dtypes
    W1 = KB + 1
    W2 = B * KB + PIX
    in_maps = []
    for core in range(N_CORES):
        coef, colsT, maskrow = _build_core_arrays(pre, tiles, slots[core], B)
        blob1 = np.empty((KB, W1), np.float32)
        blob1[:, :KB] = ustrict
        blob1[:, KB] = 1.0
        blob2 = np.empty((6, W2), np.float32)
        blob2[:, :B * KB] = coef
        blob2[:, B * KB:] = basis
        # maskrow here is [1, B*KB] with m[b] replicated; compress to [1, B]
        maskB = np.ascontiguousarray(maskrow[:, ::KB])
        in_maps.append(dict(blob1=blob1, blob2=blob2,
                            cols=colsT.astype(ml_dtypes.bfloat16),
                            maskrow=maskB))

    if B not in _compiled_cache:
        _compiled_cache[B] = _build_program(B)
    nc = _compiled_cache[B]

    from concourse.bass_utils import run_bass_kernel_spmd
    trace = bool(int(os.environ.get("GR_TRACE", "0")))
    res = run_bass_kernel_spmd(nc, in_maps, list(range(N_CORES)), trace=trace)
    if trace:
        kernel.last_exec_time_ns = res.exec_time_ns
        kernel.last_profile = res.profile_json

    out = np.zeros((3, H, W), np.float32)
    for core in range(N_CORES):
        orgb = res.results[core]["orgb"]
        ocarry = res.results[core]["ocarry"]
        seen = {}
        for s, (k, b, is_first) in enumerate(slots[core]):
            if k < 0:
                continue
            seen.setdefault(k, []).append(s)
        for k, ss in seen.items():
            ti, tj, _ = tiles[k]
            rgb = np.zeros((3, PIX), np.float32)
            for s in ss:
                rgb += orgb[:, s * PIX:(s + 1) * PIX]
            tfin = np.exp(ocarry[0, ss[-1] * PIX:(ss[-1] + 1) * PIX])
            rgb = rgb + tfin[None, :] * bg[:, None]
            out[:, ti * TS:(ti + 1) * TS, tj * TS:(tj + 1) * TS] = \
                rgb.reshape(3, TS, TS)
    return out

